# revision 6
# baseline (speedup 1.0000x reference)
"""Trainium2 Bass kernel for nn_CustomGate: y = (I_64 (x) M (x) I_64) @ x.

Math: viewing x as (a=64, j=64, r=64, b=128), the gate is
    y[a,i,r,b] = sum_j M[i,j] * x[a,j,r,b]      (complex, M is 64x64)

Complex arithmetic is folded into one real 128x128 stationary weight
    W = [[Mr^T,  Mi^T ],
         [-Mi^T, Mr^T ]]           (W[p,i] layout, p = contraction)
with rhs columns stacked as [x_real(j=0..63); x_imag(j=0..63)] per `a`
slice, so out = W.T @ rhs gives [y_real(i); y_imag(i)] per column.

Precision: the correctness gate is rel_err < 2e-2; fp16 I/O gives
~3.6e-4 while HALVING the HBM/DMA traffic vs fp32. The kernel is
DMA-bound (~33.5 MB/core at ~420 GB/s ~= 80 us), so every other stage
needs slack:
  * matmuls are fp16 (1 PE pass, not 4); a ~3.4 us warmup burst of
    dummy matmuls on a zero pad of the weight tile trips the PE HAM
    activity monitor to 2.4 GHz before real data arrives, and 1 MiB
    chunks keep PE idle gaps under the ~3.4 us re-throttle window;
  * PSUM is drained with 1024-col copies spanning two PSUM banks
    (half the instruction + semaphore overhead of per-bank copies),
    alternating between the Vector and Scalar engines;
  * input loads ride the Sync HWDGE ring, output stores the Scalar
    ring, and the last a-slice's stores move to the by-then-idle Sync
    ring so the tail drains on both rings.

Sharding: the leading `a` axis (untouched by the contraction) is split
8 ways -> 8 a-values per core.
"""

import numpy as np

import concourse.bacc as bacc
import concourse.mybir as mybir
import concourse.tile as tile
from concourse.bass_utils import run_bass_kernel_spmd

DIM = 64
WIRES = 3
BATCH = 128
D = DIM**WIRES          # 262144
N_CORES = 8
A_PER_CORE = DIM // N_CORES     # 8 a-values per core
FREE = DIM * BATCH      # 8192 elements per (a, j) row
P = 128
MM_N = 512              # moving-operand max per matmul
CP_N = 1024             # PSUM-drain copy width (2 banks)
WPAD = 512              # zero pad after W for PE warmup matmuls

_cached = {}


def _build_nc():
    f16 = mybir.dt.float16
    f32 = mybir.dt.float32
    nc = bacc.Bacc("TRN2", target_bir_lowering=False, debug=False,
                   num_devices=N_CORES)
    xs = nc.dram_tensor("xs", [A_PER_CORE, P, FREE], f16,
                        kind="ExternalInput").ap()
    w = nc.dram_tensor("w", [P, P + WPAD], f16, kind="ExternalInput").ap()
    ys = nc.dram_tensor("ys", [A_PER_CORE, P, FREE], f16,
                        kind="ExternalOutput").ap()

    with tile.TileContext(nc) as tc:
        with (
            tc.tile_pool(name="wpool", bufs=1) as wpool,
            tc.tile_pool(name="inpool", bufs=6) as inpool,
            tc.tile_pool(name="outpool", bufs=6) as outpool,
            tc.tile_pool(name="pspool", bufs=4, space="PSUM") as pspool,
        ):
            wt = wpool.tile([P, P + WPAD], f16)
            # weight load off the Sync engine so the first bulk input
            # DMA issues as early as possible
            nc.gpsimd.dma_start(wt[:], w[:, :])

            # ~3.4 us of dummy matmuls on the zero pad: trips the HAM
            # activity window so the PE is at 2.4 GHz when real chunks
            # arrive (idle/cold default is 1.2 GHz).
            for i in range(8):
                psw = pspool.tile([P, CP_N], f32, tag="ps")
                nc.tensor.matmul(psw[:, :MM_N], wt[:, :P],
                                 wt[:, P:P + MM_N], start=True, stop=True)

            chunks = []  # (a, f0, fch)
            for a in range(A_PER_CORE):
                if a == 0:
                    split = [512, 512, 1024, 2048, 4096]
                elif a == A_PER_CORE - 1:
                    split = [4096, 2048, 1024, 512, 512]
                else:
                    split = [4096, 4096]
                f0 = 0
                for fch in split:
                    chunks.append((a, f0, fch))
                    f0 += fch
                assert f0 == FREE

            ncopy = 0
            for a, f0, fch in chunks:
                xt = inpool.tile([P, fch], f16, tag="xt")
                nc.sync.dma_start(xt[:], xs[a, :, f0:f0 + fch])
                yt = outpool.tile([P, fch], f16, tag="yt")
                for c0 in range(0, fch, CP_N):
                    cw = min(CP_N, fch - c0)
                    ps = pspool.tile([P, CP_N], f32, tag="ps")
                    for k0 in range(0, cw, MM_N):
                        nc.tensor.matmul(
                            ps[:, k0:k0 + MM_N], wt[:, :P],
                            xt[:, c0 + k0:c0 + k0 + MM_N],
                            start=True, stop=True)
                    dst = yt[:, c0:c0 + cw]
                    if ncopy % 2:
                        nc.scalar.copy(dst, ps[:, :cw])
                    else:
                        nc.vector.tensor_copy(dst, ps[:, :cw])
                    ncopy += 1
                # Stores ride the Scalar HWDGE ring so their sem-waits
                # don't block the Sync ring's input loads; the final
                # a-slice's stores go on the Sync ring instead -- all
                # loads are issued by then, so the tail drains at full
                # rate on both rings.
                if a == A_PER_CORE - 1:
                    nc.sync.dma_start(ys[a, :, f0:f0 + fch], yt[:])
                else:
                    nc.scalar.dma_start(ys[a, :, f0:f0 + fch], yt[:])

    nc.compile()
    return nc


def _get_nc():
    if "nc" not in _cached:
        _cached["nc"] = _build_nc()
    return _cached["nc"]


def kernel(M_real, M_imag, x_real, x_imag, **run_kwargs):
    M_real = np.ascontiguousarray(np.asarray(M_real, dtype=np.float32))
    M_imag = np.ascontiguousarray(np.asarray(M_imag, dtype=np.float32))
    x_real = np.asarray(x_real, dtype=np.float32)
    x_imag = np.asarray(x_imag, dtype=np.float32)

    # Stationary weight W[p, i] (see module docstring) + zero warmup pad.
    W = np.zeros((P, P + WPAD), dtype=np.float16)
    W[:, :P] = np.block([[M_real.T, M_imag.T],
                         [-M_imag.T, M_real.T]]).astype(np.float16)

    # Interleave real/imag along the partition axis: xs[a, 0:64, f] = real,
    # xs[a, 64:128, f] = imag, with f = r*128 + b.
    xs_all = np.empty((DIM, P, FREE), dtype=np.float16)
    xs_all[:, :DIM, :] = x_real.reshape(DIM, DIM, FREE)
    xs_all[:, DIM:, :] = x_imag.reshape(DIM, DIM, FREE)

    nc = _get_nc()
    in_maps = [
        {"xs": xs_all[c * A_PER_CORE:(c + 1) * A_PER_CORE], "w": W}
        for c in range(N_CORES)
    ]
    r = run_bass_kernel_spmd(nc, in_maps, list(range(N_CORES)), **run_kwargs)
    if run_kwargs:
        _cached["last_result"] = r
    results = r.results

    ys_all = np.concatenate([results[c]["ys"] for c in range(N_CORES)], axis=0)
    y_real = ys_all[:, :DIM, :].reshape(D, BATCH).astype(np.float32)
    y_imag = ys_all[:, DIM:, :].reshape(D, BATCH).astype(np.float32)
    return (y_real + 1j * y_imag).astype(np.complex64)
